# revision 27
# baseline (speedup 1.0000x reference)
"""Trainium2 Bass kernel for nn_CTRule (temporal KG scoring model).

Computes, for each of B=1024 queries (h, r, t):
  v = f(E0[h], E1[r], time tables, rule tables)   # [B, 128] elementwise algebra
  scores = v @ E0.T                               # [B, 40000]

Distribution over the 8 NeuronCores (pair-hybrid): the two cores sharing each
HBM stack process the same PAIR of 128-example batch tiles but disjoint halves
of the 40000-entity axis, so each stack streams the bf16 entity table E0T only
half as much as a fully data-parallel split would:
  * each core gathers (indirect DMA) both batch tiles' embedding rows and runs
    one batched elementwise head over [128, 2, 128] on VectorE, transposing
    each v on TensorE,
  * streams its half of E0T (20000 cols, 5.12MB bf16) and matmuls both vT
    blocks against it (paired matmuls share one [128,1024] PSUM tile so each
    PSUM->SBUF copy moves 1024 columns),
  * writes a [256, 20000] bf16 block of the scores.
No cross-core communication; the host reassembles the 8 blocks and casts f32.
"""

import numpy as np
import ml_dtypes

P = 128
B = 1024
RANK = 128
NENT = 40000
NREL = 230
NTIME = 365
NBASE = 4
CYCLE = 120
NCORES = 8
NHALF = NENT // 2        # entity columns per core = 20000
CHUNK = 512              # PSUM bank width in f32
PAIRW = 2 * CHUNK        # paired-matmul copy width
LOADCH = 5000            # E0T load-chunk columns (4 loads of 1.28MB)
OUTCH = 2500             # output DMA chunk columns (8 DMAs of 640KB per tile)

# concatenated-table row widths
RC_W = 2 * RANK + 2      # [E1 | rule_C | rule_S | has_rules] = 258
TC_W = 3 * RANK          # [E4 | E2 | E3] = 384
BC_W = 2 * RANK          # [E5 | E6] = 256

TRACE = False            # set by test harness for profiling runs
_CACHE = {}


def _build():
    import concourse.bass as bass
    import concourse.mybir as mybir
    import concourse.tile as tile
    from concourse import bacc
    from concourse.masks import make_identity

    dt = mybir.dt
    mult = mybir.AluOpType.mult
    add = mybir.AluOpType.add
    sub = mybir.AluOpType.subtract

    nc = bacc.Bacc("TRN2", target_bir_lowering=False, debug=False,
                   num_devices=NCORES)

    IDX = nc.dram_tensor("IDX", [P, 8], dt.int32, kind="ExternalInput").ap()
    E0 = nc.dram_tensor("E0", [NENT, RANK], dt.float32, kind="ExternalInput").ap()
    RCAT = nc.dram_tensor("RCAT", [NREL, RC_W], dt.float32, kind="ExternalInput").ap()
    TCAT = nc.dram_tensor("TCAT", [NTIME, TC_W], dt.float32, kind="ExternalInput").ap()
    BCAT = nc.dram_tensor("BCAT", [NBASE, BC_W], dt.float32, kind="ExternalInput").ap()
    E0T = nc.dram_tensor("E0T", [RANK, NHALF], dt.bfloat16, kind="ExternalInput").ap()
    OUT = nc.dram_tensor("OUT", [2 * P, NHALF], dt.bfloat16, kind="ExternalOutput").ap()

    with tile.TileContext(nc) as tc:
        with (
            tc.tile_pool(name="const", bufs=1) as constp,
            tc.tile_pool(name="gath", bufs=1) as gp,
            tc.tile_pool(name="ew", bufs=1) as ew,
            tc.tile_pool(name="pst", bufs=1, space="PSUM") as pst,
            tc.tile_pool(name="psm", bufs=6, space="PSUM") as psm,
        ):
            # ---- index load on gpsimd's own SWDGE path so the gather chain
            # never waits on another engine's DMA ring
            idxt = gp.tile([P, 8], dt.int32)
            nc.gpsimd.dma_start(idxt[:], IDX[:])

            # ---- 4 indirect gathers per batch tile into [P, 2, W] tiles,
            # interleaved with the E0T half-table stream on the same SWDGE
            # ring: tile 0's gather packets drain first, then the first E0T
            # chunks, then tile 1's gathers, then the rest of E0T.
            lhs = gp.tile([P, 2, RANK], dt.float32)
            r8 = gp.tile([P, 2, RC_W], dt.float32)
            t8 = gp.tile([P, 2, TC_W], dt.float32)
            b8 = gp.tile([P, 2, BC_W], dt.float32)
            e0t = constp.tile([RANK, NHALF], dt.bfloat16)
            for j in range(2):
                for dst, src, col in (
                    (t8, TCAT, 2), (b8, BCAT, 3), (r8, RCAT, 1), (lhs, E0, 0),
                ):
                    nc.gpsimd.indirect_dma_start(
                        out=dst[:, j, :], out_offset=None, in_=src[:],
                        in_offset=bass.IndirectOffsetOnAxis(
                            ap=idxt[:, 4 * j + col:4 * j + col + 1],
                            axis=0))
            for c0 in range(0, NHALF, LOADCH):
                nc.gpsimd.dma_start(e0t[:, c0:c0 + LOADCH],
                                    E0T[:, c0:c0 + LOADCH])

            ident = constp.tile([P, P], dt.float32)
            make_identity(nc, ident[:])

            def TT(out, a, b_, op):
                nc.vector.tensor_tensor(out=out, in0=a, in1=b_, op=op)

            def h0(x):
                return x[:, :, 0:64]

            def h1(x):
                return x[:, :, 64:128]

            # ---- per-tile elementwise head (tile 0 first, so its matmuls
            # and output stream start as early as possible)
            TM = ew.tile([P, RANK], dt.float32)
            TE = ew.tile([P, RANK], dt.float32)
            A = ew.tile([P, RANK], dt.float32)
            Bt = ew.tile([P, RANK], dt.float32)
            Sg = ew.tile([P, RANK], dt.float32)
            Dg = ew.tile([P, RANK], dt.float32)
            t0 = ew.tile([P, 64], dt.float32)
            t1 = ew.tile([P, 64], dt.float32)
            nrs = ew.tile([P, 1], dt.float32)

            vts = []
            for j in range(2):
                REL = r8[:, j, 0:128]
                RL0, RL1 = r8[:, j, 0:64], r8[:, j, 64:128]
                RC0, RC1 = r8[:, j, 128:192], r8[:, j, 192:256]
                RS = r8[:, j, 256:257]
                HR = r8[:, j, 257:258]
                CT = t8[:, j, 0:128]
                CT0, CT1 = t8[:, j, 0:64], t8[:, j, 64:128]
                E2g, B5 = t8[:, j, 128:256], b8[:, j, 0:128]
                E3g, B6 = t8[:, j, 256:384], b8[:, j, 128:256]
                LHS = lhs[:, j, 0:128]
                L0, L1 = lhs[:, j, 0:64], lhs[:, j, 64:128]
                V = ew.tile([P, RANK], dt.float32, name=f"V{j}")

                def g0(x):
                    return x[:, 0:64]

                def g1(x):
                    return x[:, 64:128]

                # time = E2[t] + E5[tb];  time_ent = E3[t] + E6[tb]
                TT(TM[:], E2g, B5, add)
                TT(TE[:], E3g, B6, add)
                nc.vector.tensor_scalar_mul(nrs[:], RS, -1.0)
                # A = cmul(comp_time, rule_C[r])
                TT(t0[:], CT0, RC0, mult)
                TT(t1[:], CT1, RC1, mult)
                TT(g0(A), t0[:], t1[:], sub)
                TT(t0[:], CT0, RC1, mult)
                TT(t1[:], CT1, RC0, mult)
                TT(g1(A), t0[:], t1[:], add)
                # A = rule_branch = A - rule_S*rel (fused: A = rel*(-RS) + A)
                nc.vector.scalar_tensor_tensor(
                    out=A[:], in0=REL, scalar=nrs[:], in1=A[:],
                    op0=mult, op1=add)
                # Bt = cmul(rel, lhs); then Bt = norule_branch = lhs + Bt
                TT(t0[:], RL0, L0, mult)
                TT(t1[:], RL1, L1, mult)
                TT(g0(Bt), t0[:], t1[:], sub)
                TT(t0[:], RL0, L1, mult)
                TT(t1[:], RL1, L0, mult)
                TT(g1(Bt), t0[:], t1[:], add)
                TT(Bt[:], LHS, Bt[:], add)
                # A = rule_score = Bt + HR*(A - Bt); then A = q = CT + A
                TT(A[:], A[:], Bt[:], sub)
                nc.vector.scalar_tensor_tensor(
                    out=A[:], in0=A[:], scalar=HR, in1=Bt[:],
                    op0=mult, op1=add)
                TT(A[:], A[:], CT, add)
                # Bt = complex_mul(rel, q) = [R0*q0 + R1*q1, R0*q1 - R1*q0]
                TT(t0[:], RL0, g0(A), mult)
                TT(t1[:], RL1, g1(A), mult)
                TT(g0(Bt), t0[:], t1[:], add)
                TT(t0[:], RL0, g1(A), mult)
                TT(t1[:], RL1, g0(A), mult)
                TT(g1(Bt), t0[:], t1[:], sub)
                # Bt = rel_ = rel + Bt ;  S = rel_ + time ; D = rel_ - time
                TT(Bt[:], Bt[:], REL, add)
                TT(Sg[:], Bt[:], TM[:], add)
                TT(Dg[:], Bt[:], TM[:], sub)
                # V0 = L0*S0 + TE0*D0 - L1*S1 + TE1*D1
                TT(t0[:], L0, g0(Sg), mult)
                TT(t1[:], g0(TE), g0(Dg), mult)
                TT(g0(V), t0[:], t1[:], add)
                TT(t0[:], L1, g1(Sg), mult)
                TT(g0(V), g0(V), t0[:], sub)
                TT(t1[:], g1(TE), g1(Dg), mult)
                TT(g0(V), g0(V), t1[:], add)
                # V1 = L1*S0 + L0*S1 + TE1*D0 - TE0*D1
                TT(t0[:], L1, g0(Sg), mult)
                TT(t1[:], L0, g1(Sg), mult)
                TT(g1(V), t0[:], t1[:], add)
                TT(t0[:], g1(TE), g0(Dg), mult)
                TT(g1(V), g1(V), t0[:], add)
                TT(t1[:], g0(TE), g1(Dg), mult)
                TT(g1(V), g1(V), t1[:], sub)

                # transpose + bf16 cast of vT (stationary matmul operand)
                vt_ps = pst.tile([P, P], dt.float32, space="PSUM", tag="vtps")
                nc.tensor.transpose(out=vt_ps[:], in_=V[:], identity=ident[:])
                vt = constp.tile([P, P], dt.bfloat16, name=f"vt{j}")
                nc.scalar.copy(out=vt[:], in_=vt_ps[:])
                vts.append(vt)

            # ---- stream matmuls: both batch tiles over this core's half.
            # Pairs of 512-wide matmuls share one [128,1024] PSUM tile so each
            # PSUM->SBUF copy moves 1024 columns.
            for j in range(2):
                vt = vts[j]
                osb = constp.tile([P, NHALF], dt.bfloat16, name=f"osb{j}")
                next_out = OUTCH
                for c0 in range(0, NHALF, CHUNK):
                    cw = min(CHUNK, NHALF - c0)
                    mm = psm.tile([P, CHUNK], dt.float32, space="PSUM", tag="mm")
                    nc.tensor.matmul(out=mm[:, :cw], lhsT=vt[:],
                                     rhs=e0t[:, c0:c0 + cw],
                                     start=True, stop=True)
                    if (c0 // CHUNK) % 2 == 0:
                        nc.scalar.copy(out=osb[:, c0:c0 + cw], in_=mm[:, :cw])
                    else:
                        nc.vector.tensor_copy(out=osb[:, c0:c0 + cw],
                                              in_=mm[:, :cw])
                    if c0 + cw >= next_out:
                        o0 = next_out - OUTCH
                        nc.sync.dma_start(OUT[j * P:(j + 1) * P, o0:next_out],
                                          osb[:, o0:next_out])
                        next_out += OUTCH

    nc.compile()
    return nc


def _prep_inputs(inputs):
    x = np.asarray(inputs["x"])
    E0 = np.ascontiguousarray(np.asarray(inputs["E0"], dtype=np.float32))
    E1 = np.asarray(inputs["E1"], dtype=np.float32)
    E2 = np.asarray(inputs["E2"], dtype=np.float32)
    E3 = np.asarray(inputs["E3"], dtype=np.float32)
    E4 = np.asarray(inputs["E4"], dtype=np.float32)
    E5 = np.asarray(inputs["E5"], dtype=np.float32)
    E6 = np.asarray(inputs["E6"], dtype=np.float32)
    rule_C = np.asarray(inputs["rule_C"], dtype=np.float32)
    rule_S = np.asarray(inputs["rule_S"], dtype=np.float32)
    has_rules = np.asarray(inputs["has_rules"])

    idx = np.empty((B, 4), np.int32)
    idx[:, 0] = x[:, 0]
    idx[:, 1] = x[:, 1]
    idx[:, 2] = x[:, 3]
    idx[:, 3] = x[:, 3] // CYCLE

    rcat = np.concatenate(
        [E1, rule_C, rule_S[:, None],
         has_rules.astype(np.float32)[:, None]], axis=1).astype(np.float32)
    tcat = np.ascontiguousarray(np.concatenate([E4, E2, E3], axis=1))
    bcat = np.ascontiguousarray(np.concatenate([E5, E6], axis=1))
    e0t = np.ascontiguousarray(E0.T).astype(ml_dtypes.bfloat16)
    e0t_halves = [np.ascontiguousarray(e0t[:, :NHALF]),
                  np.ascontiguousarray(e0t[:, NHALF:])]

    in_maps = []
    for c in range(NCORES):
        p = c // 2
        idx2 = np.hstack([idx[2 * p * P:(2 * p + 1) * P],
                          idx[(2 * p + 1) * P:(2 * p + 2) * P]])
        in_maps.append({
            "IDX": np.ascontiguousarray(idx2),
            "E0": E0, "RCAT": rcat, "TCAT": tcat, "BCAT": bcat,
            "E0T": e0t_halves[c % 2],
        })
    return in_maps


def kernel(**inputs):
    from concourse.bass_utils import run_bass_kernel_spmd

    if "nc" not in _CACHE:
        _CACHE["nc"] = _build()
    nc = _CACHE["nc"]

    in_maps = _prep_inputs(inputs)
    res = run_bass_kernel_spmd(nc, in_maps, core_ids=list(range(NCORES)),
                               trace=TRACE)
    _CACHE["last_result"] = res
    out = np.empty((B, NENT), np.float32)
    for p in range(NCORES // 2):
        lo = res.results[2 * p]["OUT"]        # [256, 0:20000]
        hi = res.results[2 * p + 1]["OUT"]    # [256, 20000:40000]
        rows = slice(2 * p * P, (2 * p + 2) * P)
        out[rows, :NHALF] = lo
        out[rows, NHALF:] = hi
    return out
